# revision 2
# baseline (speedup 1.0000x reference)
"""Trainium2 Bass kernel for nn_BaseTBLoRa (moe_routing).

out[b,s,:] = x[b,s,:] @ W_base.T + b_base + 2.0 * ((x[b,s,:] @ A_w[e_b].T) @ B_w[e_b].T)
with e_b = segment[b].  B=8, S=2048, D=1024, Do=1024, R=16, E=8.

Sharding: data-parallel over batch — core b handles batch b (B == n_cores).

Key transformation vs the earlier version: the LoRA branch is merged into the
base weight on the host (standard LoRA weight-merge):
    W_eff[e] = W_base + LORA_SCALE * B_w[e] @ A_w[e]          # [Do, D]
which is mathematically identical to the reference's selected branch. Each
core then runs a single dense GEMM
    y = x_b @ W_eff[e_b].T + b_base
with no on-device LoRA matmuls at all (removes ~25% of the PE stream time:
the h = x@A^T chunk matmuls and the K-padded h@B^T matmuls are gone).

Device math per core (bf16 matmuls, fp32 PSUM accumulation):
  for each 512-wide s macro-chunk (4 of them):
    for each 128-row s-tile (4 per chunk):
      ps_y[s128, 0:512]    = sum_dt xT[dt, s128].T @ WT[dt, 0:512]    (8 MMs, N=512)
      ps_y[s128, 512:1024] = sum_dt xT[dt, s128].T @ WT[dt, 512:]     (8 MMs, N=512)
      o_bf16 = DVE tensor_add(ps_y, bias_rep)   # fused bias add + bf16 cast
      DMA o_bf16 to DRAM (host upconverts to fp32)

  - 256 MMs of N=512 bf16 @ ~215-250ns back-to-back => ~57-64us PE floor.
  - bias is added by the DVE during the PSUM->SBUF drain (bias replicated
    across the 128 partitions host-side), so it costs no PE time.
  - output is DMA'd as bf16 (2KB/partition lines), halving store traffic;
    the host converts back to fp32. Rounding adds <=2^-9*|y| error, well
    inside the 2e-2 gate.
"""

import ml_dtypes
import numpy as np

import concourse.tile as tile
from concourse import bacc, mybir
from concourse.bass_utils import run_bass_kernel_spmd

LORA_SCALE = 32.0 / 16.0

B, S, D, DO, R = 8, 2048, 1024, 1024, 16
NDT = D // 128   # 8 contraction tiles
NST = S // 128   # 16 s-tiles
NSC = 4          # s macro-chunks
SC = S // NSC    # 512 s per macro-chunk
SUB = SC // 128  # 4 s-tiles per macro-chunk
N_CORES = 8

F32 = mybir.dt.float32
BF16 = mybir.dt.bfloat16

last_in_maps = None
last_results = None


def _build(loop_n=0):
    """loop_n > 0 wraps the body in a dynamic For_i (used only for dilation
    timing); the graded path uses loop_n=0 (straight-line program)."""
    import contextlib

    nc = bacc.Bacc("TRN2", target_bir_lowering=False, debug=False)

    x_d = nc.dram_tensor("x5", [NSC, 128, NDT, SC], BF16, kind="ExternalInput")
    w_d = nc.dram_tensor("wt", [NDT, 128, DO], BF16, kind="ExternalInput")
    bias_d = nc.dram_tensor("bias", [128, DO], F32, kind="ExternalInput")
    out_d = nc.dram_tensor("out", [NST, 128, DO], BF16, kind="ExternalOutput")

    with tile.TileContext(nc) as tc:
        with (
            tc.tile_pool(name="wpool", bufs=1) as wpool,
            tc.tile_pool(name="cpool", bufs=1) as cpool,
            tc.tile_pool(name="xpool", bufs=3) as xpool,
            tc.tile_pool(name="opool", bufs=4) as opool,
            tc.tile_pool(name="psy", bufs=3, space="PSUM") as psy,
        ):
            loop_cm = tc.For_i(0, loop_n, 1) if loop_n else contextlib.nullcontext()

            def preload():
                bias_t = cpool.tile([128, DO], F32)
                nc.sync.dma_start(bias_t[:], bias_d[:])
                w_t = wpool.tile([128, NDT, DO], BF16)
                return bias_t, w_t

            # A For_i body may not touch tiles allocated outside the loop, so
            # in timing mode the preload moves inside (slightly conservative).
            if not loop_n:
                bias_t, w_t = preload()
            with loop_cm:
                if loop_n:
                    bias_t, w_t = preload()
                for sc in range(NSC):
                    x_t = xpool.tile([128, NDT, SC], BF16)
                    for dt in range(NDT):
                        nc.sync.dma_start(x_t[:, dt, :], x_d[sc, :, dt, :])
                        if sc == 0:
                            # interleave the W preload with the first x chunk
                            nc.sync.dma_start(w_t[:, dt, :], w_d[dt])

                    for sub in range(SUB):
                        st = sc * SUB + sub
                        ps_y = psy.tile([128, DO], F32)
                        for dt in range(NDT):
                            xt = x_t[:, dt, sub * 128:(sub + 1) * 128]
                            first = dt == 0
                            last = dt == NDT - 1
                            nc.tensor.matmul(
                                ps_y[:, 0:512], xt, w_t[:, dt, 0:512],
                                start=first, stop=last,
                            )
                            nc.tensor.matmul(
                                ps_y[:, 512:1024], xt, w_t[:, dt, 512:1024],
                                start=first, stop=last,
                            )

                        o_t = opool.tile([128, DO], BF16)
                        # fused bias add + fp32->bf16 cast during PSUM drain
                        nc.vector.tensor_add(o_t[:], ps_y[:], bias_t[:])
                        nc.sync.dma_start(out_d[st], o_t[:])

    nc.compile()
    return nc


def _prep_core_inputs(x_b, wt5, bias_rep):
    xT = x_b.T.reshape(NDT, 128, NSC, SC).transpose(2, 1, 0, 3).astype(
        ml_dtypes.bfloat16
    )
    return {"x5": xT, "wt": wt5, "bias": bias_rep}


def kernel(x, segment, W_base, b_base, A_w, B_w, _sim=False):
    global last_in_maps, last_results

    x = np.asarray(x, dtype=np.float32)
    W_base = np.asarray(W_base, dtype=np.float32)
    b_base = np.asarray(b_base, dtype=np.float32)
    A_w = np.asarray(A_w, dtype=np.float32)
    B_w = np.asarray(B_w, dtype=np.float32)
    seg = np.asarray(segment).astype(np.int64)

    # LoRA merge on host: W_eff[e] = W_base + scale * B@A, transposed and
    # tiled for the device ([NDT, 128, DO], contraction on partitions).
    wt_by_expert = {}
    for e in set(int(v) for v in seg):
        w_eff = W_base + LORA_SCALE * (B_w[e] @ A_w[e])
        wt_by_expert[e] = np.ascontiguousarray(w_eff.T).reshape(
            NDT, 128, DO).astype(ml_dtypes.bfloat16)

    bias_rep = np.ascontiguousarray(
        np.broadcast_to(b_base, (128, DO))).astype(np.float32)

    in_maps = [
        _prep_core_inputs(x[b], wt_by_expert[int(seg[b])], bias_rep)
        for b in range(B)
    ]
    last_in_maps = in_maps

    nc = _build()

    if _sim:
        from concourse.bass_interp import CoreSim

        outs = []
        for b in range(B):
            sim = CoreSim(nc)
            for name, arr in in_maps[b].items():
                sim.tensor(name)[:] = arr
            sim.simulate()
            outs.append(
                np.array(sim.tensor("out")).astype(np.float32).reshape(S, DO))
        return np.stack(outs)

    res = run_bass_kernel_spmd(nc, in_maps, list(range(N_CORES)))
    last_results = res
    return np.stack([
        np.asarray(res.results[c]["out"]).astype(np.float32).reshape(S, DO)
        for c in range(N_CORES)
    ])


# revision 6
# speedup vs baseline: 1.1099x; 1.1099x over previous
"""Trainium2 Bass kernel for nn_BaseTBLoRa (moe_routing).

out[b,s,:] = x[b,s,:] @ W_base.T + b_base + 2.0 * ((x[b,s,:] @ A_w[e_b].T) @ B_w[e_b].T)
with e_b = segment[b].  B=8, S=2048, D=1024, Do=1024, R=16, E=8.

Sharding: data-parallel over batch — core b handles batch b (B == n_cores).
No collectives; each core's output slice is gathered on the host.

Key transformation vs the earlier on-device-LoRA version: the LoRA branch is
merged into the base weight on the host (standard LoRA weight-merge, exact
algebra):
    W_eff[e] = W_base + LORA_SCALE * B_w[e] @ A_w[e]          # [Do, D]
so each core runs a single dense GEMM
    y = x_b @ W_eff[e_b].T + b_base
with no on-device LoRA matmuls at all. This removes the h = x@A^T chunk
matmuls and the K-padded h@B^T matmuls (~20% of the PE stream time, HW
A/B-measured: 115.3us -> 101.8us per iteration in the same session).

Device program per core (bf16 matmuls, fp32 PSUM accumulation):
  preload: bias (bf16, replicated over partitions) via 1 DMA
  for each 512-wide s macro-chunk (4 of them):
    x chunk DMA'd per k-tile (8 x 128KB); W interleaved with chunk 0 (8 x 256KB)
    for each 128-row s-tile (4 per chunk):
      ps_y[s128, 0:512]    = sum_dt xT[dt, s128].T @ WT[dt, 0:512]    (8 MMs, N=512)
      ps_y[s128, 512:1024] = sum_dt xT[dt, s128].T @ WT[dt, 512:]     (8 MMs, N=512)
      o = DVE tensor_add(ps_y, bias_rep)    # fused bias add + bf16 cast
      DMA o to DRAM (256KB)

Measured/modeled design points:
  - 256 MMs of N=512 bf16: 512 cyc @ 2.4 GHz = 213 ns/MM back-to-back;
    54.6 us PE-stream floor (cost-model total 66.6 us incl. DMA lead-in and
    p-state ramp; HW dilation ~85-100 us/iter depending on thermal state).
  - Fine-grained DMA beats batched 1MB DMAs on HW AND in the cost model
    (91.5 vs 86.1 us HW same-session): per-k-tile pieces keep the first MM
    group fed progressively; batching delays the pipeline more than the
    per-descriptor fixed cost saves.
  - bias add rides the DVE during the PSUM->SBUF drain (free: DVE has 3x
    slack vs PE); bias is bf16 to halve its preload DMA.
  - output DMA'd as bf16 (halves store traffic); host upconverts to fp32.
    Adds <=2^-9*|y| rounding; total rel err 3.3e-3 vs the 2e-2 gate.
"""

import ml_dtypes
import numpy as np

import concourse.tile as tile
from concourse import bacc, mybir
from concourse.bass_utils import run_bass_kernel_spmd

LORA_SCALE = 32.0 / 16.0

B, S, D, DO, R = 8, 2048, 1024, 1024, 16
NDT = D // 128   # 8 contraction tiles
NST = S // 128   # 16 s-tiles
NSC = 4          # s macro-chunks
SC = S // NSC    # 512 s per macro-chunk
SUB = SC // 128  # 4 s-tiles per macro-chunk
N_CORES = 8

F32 = mybir.dt.float32
BF16 = mybir.dt.bfloat16

last_in_maps = None
last_results = None


def _build(loop_n=0):
    """loop_n > 0 wraps the body in a dynamic For_i (used only for dilation
    timing); the graded path uses loop_n=0 (straight-line program)."""
    import contextlib

    nc = bacc.Bacc("TRN2", target_bir_lowering=False, debug=False)

    x_d = nc.dram_tensor("x5", [NSC, 128, NDT, SC], BF16, kind="ExternalInput")
    w_d = nc.dram_tensor("wt", [128, NDT, DO], BF16, kind="ExternalInput")
    bias_d = nc.dram_tensor("bias", [128, DO], BF16, kind="ExternalInput")
    out_d = nc.dram_tensor("out", [NST, 128, DO], BF16, kind="ExternalOutput")

    with tile.TileContext(nc) as tc:
        with (
            tc.tile_pool(name="wpool", bufs=1) as wpool,
            tc.tile_pool(name="cpool", bufs=1) as cpool,
            tc.tile_pool(name="xpool", bufs=3) as xpool,
            tc.tile_pool(name="opool", bufs=4) as opool,
            tc.tile_pool(name="psy", bufs=3, space="PSUM") as psy,
        ):
            loop_cm = tc.For_i(0, loop_n, 1) if loop_n else contextlib.nullcontext()

            def preload():
                bias_t = cpool.tile([128, DO], BF16)
                nc.sync.dma_start(bias_t[:], bias_d[:])
                w_t = wpool.tile([128, NDT, DO], BF16)
                return bias_t, w_t

            # A For_i body may not touch tiles allocated outside the loop, so
            # in timing mode the preload moves inside (slightly conservative).
            if not loop_n:
                bias_t, w_t = preload()
            with loop_cm:
                if loop_n:
                    bias_t, w_t = preload()
                for sc in range(NSC):
                    x_t = xpool.tile([128, NDT, SC], BF16)
                    for dt in range(NDT):
                        nc.sync.dma_start(x_t[:, dt, :], x_d[sc, :, dt, :])
                        if sc == 0:
                            # interleave the W preload with the first x chunk
                            nc.sync.dma_start(w_t[:, dt, :], w_d[:, dt, :])

                    for sub in range(SUB):
                        st = sc * SUB + sub
                        ps_y = psy.tile([128, DO], F32)
                        for dt in range(NDT):
                            xt = x_t[:, dt, sub * 128:(sub + 1) * 128]
                            first = dt == 0
                            last = dt == NDT - 1
                            nc.tensor.matmul(
                                ps_y[:, 0:512], xt, w_t[:, dt, 0:512],
                                start=first, stop=last,
                            )
                            nc.tensor.matmul(
                                ps_y[:, 512:1024], xt, w_t[:, dt, 512:1024],
                                start=first, stop=last,
                            )

                        o_t = opool.tile([128, DO], BF16)
                        # fused bias add + fp32->bf16 cast during PSUM drain
                        nc.vector.tensor_add(o_t[:], ps_y[:], bias_t[:])
                        nc.sync.dma_start(out_d[st], o_t[:])

    nc.compile()
    return nc


def _prep_core_inputs(x_b, wt5, bias_rep):
    xT = x_b.T.reshape(NDT, 128, NSC, SC).transpose(2, 1, 0, 3).astype(
        ml_dtypes.bfloat16
    )
    return {"x5": xT, "wt": wt5, "bias": bias_rep}


def make_in_maps(x, seg, W, b, A_w, B_w):
    """Host-side prep for all 8 cores (used by kernel() and timing scripts)."""
    wt_by_expert = {}
    for e in set(int(v) for v in seg):
        w_eff = W + LORA_SCALE * (B_w[e] @ A_w[e])
        wt_by_expert[e] = np.ascontiguousarray(
            w_eff.T.reshape(NDT, 128, DO).transpose(1, 0, 2)
        ).astype(ml_dtypes.bfloat16)
    bias_rep = np.ascontiguousarray(
        np.broadcast_to(b, (128, DO))).astype(ml_dtypes.bfloat16)
    return [
        _prep_core_inputs(x[bb], wt_by_expert[int(seg[bb])], bias_rep)
        for bb in range(B)
    ]


def kernel(x, segment, W_base, b_base, A_w, B_w, _sim=False):
    global last_in_maps, last_results

    x = np.asarray(x, dtype=np.float32)
    W_base = np.asarray(W_base, dtype=np.float32)
    b_base = np.asarray(b_base, dtype=np.float32)
    A_w = np.asarray(A_w, dtype=np.float32)
    B_w = np.asarray(B_w, dtype=np.float32)
    seg = np.asarray(segment).astype(np.int64)

    in_maps = make_in_maps(x, seg, W_base, b_base, A_w, B_w)
    last_in_maps = in_maps

    nc = _build()

    if _sim:
        from concourse.bass_interp import CoreSim

        outs = []
        for b in range(B):
            sim = CoreSim(nc)
            for name, arr in in_maps[b].items():
                sim.tensor(name)[:] = arr
            sim.simulate()
            outs.append(
                np.asarray(sim.tensor("out")).astype(np.float32).reshape(S, DO))
        return np.stack(outs)

    res = run_bass_kernel_spmd(nc, in_maps, list(range(N_CORES)))
    last_results = res
    return np.stack([
        np.asarray(res.results[c]["out"]).astype(np.float32).reshape(S, DO)
        for c in range(N_CORES)
    ])
